# revision 10
# baseline (speedup 1.0000x reference)
"""Trainium2 Bass kernel for nn_MentionScore (segment_reduce).

Contract: kernel(**inputs) takes the FULL unsharded inputs (as produced by
reference.setup_inputs()) and returns the FULL output (g_i, mention_scores),
matching reference.reference(**inputs).

Sharding: spans are data-parallel across 8 NeuronCores (20000 spans each,
padded to 20480).  Each core receives only its token window of states/embeds
(starts are sorted, so each core's spans touch a contiguous ~10k-token range)
plus local span indices.  All compute (attention MLP, span softmax, gather,
mention MLP) runs on-device:

  Phase A (per core): dense attention-MLP over the token window in a
    feature-major layout (PE transposes), fused exp -> e=exp(alfa); e-pairs
    (e[t], e[t+1]) written to a small DRAM scratch.
  Phase B (per core): for each 128-span tile, indirect-DMA row gathers
    (states@start, states@end land directly in the g_i tile; embeds@start/end;
    e-pairs), DVE ops for span-softmax weights / attention embeds / width
    embedding, g_i tile DMA'd out; PE transposes of g_i feed the mention MLP
    (float32r matmuls, N=512 groups), scores DMA'd out at the end.
"""

import numpy as np

import concourse.bass as bass
import concourse.bacc as bacc
import concourse.mybir as mybir
import concourse.tile as tile
from concourse.bass_utils import run_bass_kernel_spmd
from concourse.masks import make_identity

F32 = mybir.dt.float32
F32R = mybir.dt.float32r
I32 = mybir.dt.int32

N_CORES = 8
P = 128

N_T = 80000
N_S = 160000
D_S = 400          # states feature dim
D_E = 350          # embeds feature dim
D_D = 20           # width-embedding dim
H = 150            # hidden dim
HP = 160           # hidden dim padded to 128+32
GI = 1170          # 2*D_S + D_E + D_D

S_C = N_S // N_CORES           # real spans per core (20000)
TILES = 160                    # span tiles per core (padded)
S_LOC = TILES * P              # padded spans per core (20480)
GRP = 4                        # span tiles per matmul group (N=512)
N_GRP = TILES // GRP

# K-chunking of the mention MLP input dim (1170 = 9*128 + 18)
SK_CH = [(kc * P, min(P, GI - kc * P)) for kc in range((GI + P - 1) // P)]
# K-chunking of the attention MLP input dim (400 = 3*128 + 16)
AK_CH = [(kc * P, min(P, D_S - kc * P)) for kc in range((D_S + P - 1) // P)]
# M-chunking of the padded hidden dim (160 = 128 + 32)
M_CH = [(0, P), (P, HP - P)]

A_GRP_T = 512                  # tokens per alfa-phase group

# caches keyed by T_LOC so repeated calls in one process skip rebuild/compile
_NC_CACHE = {}
LAST_RUN = {}                  # introspection for test.py (nc, in_maps)


def _build(T_LOC: int, tiles: int = TILES, num_devices: int = N_CORES):
    key = (T_LOC, tiles, num_devices)
    if key in _NC_CACHE:
        return _NC_CACHE[key]
    s_loc_n = tiles * P
    n_grp = tiles // GRP

    nc = bacc.Bacc("TRN2", target_bir_lowering=False, debug=False,
                   num_devices=num_devices)

    # ---- DRAM tensors ----
    t_states = nc.dram_tensor("states_loc", [T_LOC, D_S], F32, kind="ExternalInput")
    t_embeds = nc.dram_tensor("embeds_loc", [T_LOC, D_E], F32, kind="ExternalInput")
    t_starts = nc.dram_tensor("starts_w", [P, tiles], I32, kind="ExternalInput")
    t_ends = nc.dram_tensor("ends_w", [P, tiles], I32, kind="ExternalInput")
    t_widths = nc.dram_tensor("widths_w", [P, tiles], F32, kind="ExternalInput")
    t_wt1 = nc.dram_tensor("wt1", [P, D_D], F32, kind="ExternalInput")
    t_wtd = nc.dram_tensor("wtd", [P, D_D], F32, kind="ExternalInput")
    t_aw1 = nc.dram_tensor("aw1w", [P, len(AK_CH), HP], F32, kind="ExternalInput")
    t_aw2 = nc.dram_tensor("aw2w", [P, 2, HP], F32, kind="ExternalInput")
    t_aw3 = nc.dram_tensor("aw3w", [P, 2], F32, kind="ExternalInput")
    t_ab1 = nc.dram_tensor("ab1c", [P, 2], F32, kind="ExternalInput")
    t_ab2 = nc.dram_tensor("ab2c", [P, 2], F32, kind="ExternalInput")
    t_ab3 = nc.dram_tensor("ab3", [1, 1], F32, kind="ExternalInput")
    t_sw1 = nc.dram_tensor("sw1w", [P, len(SK_CH), HP], F32, kind="ExternalInput")
    t_sw2 = nc.dram_tensor("sw2w", [P, 2, HP], F32, kind="ExternalInput")
    t_sw3 = nc.dram_tensor("sw3w", [P, 2], F32, kind="ExternalInput")
    t_sb1 = nc.dram_tensor("sb1c", [P, 2], F32, kind="ExternalInput")
    t_sb2 = nc.dram_tensor("sb2c", [P, 2], F32, kind="ExternalInput")
    t_sb3 = nc.dram_tensor("sb3", [1, 1], F32, kind="ExternalInput")

    t_gi = nc.dram_tensor("gi_out", [s_loc_n, GI], F32, kind="ExternalOutput")
    t_sc = nc.dram_tensor("sc_out", [s_loc_n, 1], F32, kind="ExternalOutput")

    t_ep = nc.dram_tensor("ep_scratch", [T_LOC, 2], F32)  # internal

    n_agrp = T_LOC // A_GRP_T

    with tile.TileContext(nc) as tc:
        with tc.tile_pool(name="const", bufs=1) as cpool, \
             tc.tile_pool(name="work", bufs=3) as wpool, \
             tc.tile_pool(name="gipool", bufs=4) as gipool, \
             tc.tile_pool(name="gitp", bufs=2) as gitpool, \
             tc.tile_pool(name="hb", bufs=2) as hpool, \
             tc.tile_pool(name="ptp", bufs=3, space="PSUM") as ptp, \
             tc.tile_pool(name="pmm", bufs=1, space="PSUM") as pmm:

            # ---- constants / params ----
            ident = cpool.tile([P, P], F32)
            make_identity(nc, ident[:])

            def load_param(nm, dram, shape, dtype=F32R, engine=None):
                t = cpool.tile(shape, dtype, name=nm, tag=nm)
                eng = engine or nc.gpsimd
                eng.dma_start(out=t[:], in_=dram[:])
                return t

            aw1 = load_param("aw1", t_aw1, [P, len(AK_CH), HP])
            aw2 = load_param("aw2", t_aw2, [P, 2, HP])
            aw3 = load_param("aw3", t_aw3, [P, 2])
            sw1 = load_param("sw1", t_sw1, [P, len(SK_CH), HP])
            sw2 = load_param("sw2", t_sw2, [P, 2, HP])
            sw3 = load_param("sw3", t_sw3, [P, 2])
            ab1 = load_param("ab1", t_ab1, [P, 2], F32, nc.sync)
            ab2 = load_param("ab2", t_ab2, [P, 2], F32, nc.sync)
            ab3 = load_param("ab3", t_ab3, [1, 1], F32, nc.sync)
            sb1 = load_param("sb1", t_sb1, [P, 2], F32, nc.sync)
            sb2 = load_param("sb2", t_sb2, [P, 2], F32, nc.sync)
            sb3 = load_param("sb3", t_sb3, [1, 1], F32, nc.sync)
            wt1 = load_param("wt1", t_wt1, [P, D_D], F32, nc.sync)
            wtd = load_param("wtd", t_wtd, [P, D_D], F32, nc.sync)
            starts_sb = load_param("starts_sb", t_starts, [P, tiles], I32, nc.sync)
            ends_sb = load_param("ends_sb", t_ends, [P, tiles], I32, nc.sync)
            widths_sb = load_param("widths_sb", t_widths, [P, tiles], F32, nc.sync)

            e_row = cpool.tile([1, T_LOC + 8], F32)
            nc.gpsimd.memset(e_row[0:1, T_LOC:], 0.0)

            # =========== Phase A: alfa MLP + exp, token-window dense ===========
            for g in range(n_agrp):
                t0 = g * A_GRP_T
                stT = wpool.tile([P, len(AK_CH), A_GRP_T], F32R, tag="stT", bufs=2)
                for st in range(A_GRP_T // P):
                    s_tile = wpool.tile([P, D_S], F32, tag="s_in")
                    nc.sync.dma_start(
                        out=s_tile[:],
                        in_=t_states[t0 + st * P: t0 + (st + 1) * P, :])
                    for kc, (k0, kw) in enumerate(AK_CH):
                        tp = ptp.tile([P, P], F32, space="PSUM", tag="tp")
                        nc.tensor.transpose(out=tp[:kw, :],
                                            in_=s_tile[:, k0:k0 + kw],
                                            identity=ident[:])
                        nc.vector.tensor_copy(
                            out=stT[0:kw, kc, st * P:(st + 1) * P],
                            in_=tp[0:kw, :])
                # L1
                h1ps = [pmm.tile([P, A_GRP_T], F32, space="PSUM", tag="hps0", name="hps0"),
                        pmm.tile([32, A_GRP_T], F32, space="PSUM", tag="hps1", name="hps1")]
                for mc, (m0, mw) in enumerate(M_CH):
                    for kc, (k0, kw) in enumerate(AK_CH):
                        nc.tensor.matmul(out=h1ps[mc][0:mw, :],
                                         lhsT=aw1[0:kw, kc, m0:m0 + mw],
                                         rhs=stT[0:kw, kc, :],
                                         start=(kc == 0),
                                         stop=(kc == len(AK_CH) - 1))
                h1sb = hpool.tile([P, A_GRP_T], F32R, tag="h1sb")
                h1sb2 = hpool.tile([32, A_GRP_T], F32R, tag="h1sb2")
                nc.scalar.activation(out=h1sb[:], in_=h1ps[0][:],
                                     func=mybir.ActivationFunctionType.Relu,
                                     bias=ab1[:, 0:1])
                nc.scalar.activation(out=h1sb2[:], in_=h1ps[1][0:32, :],
                                     func=mybir.ActivationFunctionType.Relu,
                                     bias=ab1[0:32, 1:2])
                # L2
                h2ps = [pmm.tile([P, A_GRP_T], F32, space="PSUM", tag="h2ps0", name="h2ps0"),
                        pmm.tile([32, A_GRP_T], F32, space="PSUM", tag="h2ps1", name="h2ps1")]
                for mc, (m0, mw) in enumerate(M_CH):
                    nc.tensor.matmul(out=h2ps[mc][0:mw, :],
                                     lhsT=aw2[0:P, 0, m0:m0 + mw],
                                     rhs=h1sb[:], start=True, stop=False)
                    nc.tensor.matmul(out=h2ps[mc][0:mw, :],
                                     lhsT=aw2[0:32, 1, m0:m0 + mw],
                                     rhs=h1sb2[:], start=False, stop=True)
                h2sb = hpool.tile([P, A_GRP_T], F32R, tag="h2sb")
                h2sb2 = hpool.tile([32, A_GRP_T], F32R, tag="h2sb2")
                nc.scalar.activation(out=h2sb[:], in_=h2ps[0][:],
                                     func=mybir.ActivationFunctionType.Relu,
                                     bias=ab2[:, 0:1])
                nc.scalar.activation(out=h2sb2[:], in_=h2ps[1][0:32, :],
                                     func=mybir.ActivationFunctionType.Relu,
                                     bias=ab2[0:32, 1:2])
                # L3 + exp
                scps = pmm.tile([1, A_GRP_T], F32, space="PSUM", tag="scps")
                nc.tensor.matmul(out=scps[:], lhsT=aw3[0:P, 0:1], rhs=h2sb[:],
                                 start=True, stop=False)
                nc.tensor.matmul(out=scps[:], lhsT=aw3[0:32, 1:2], rhs=h2sb2[:],
                                 start=False, stop=True)
                nc.scalar.activation(out=e_row[0:1, t0:t0 + A_GRP_T],
                                     in_=scps[:],
                                     func=mybir.ActivationFunctionType.Exp,
                                     bias=ab3[0:1, 0:1])

            # e-pairs (e[t], e[t+1]) -> DRAM scratch, in 2048-token chunks
            EPC = min(2048, T_LOC)
            for c0 in range(0, T_LOC, EPC):
                cw = min(EPC, T_LOC - c0)
                ep_g = wpool.tile([1, 2 * EPC], F32, tag="ep_g", bufs=2)
                nc.vector.tensor_copy(out=ep_g[0:1, 0:2 * cw:2],
                                      in_=e_row[0:1, c0:c0 + cw])
                nc.vector.tensor_copy(out=ep_g[0:1, 1:2 * cw:2],
                                      in_=e_row[0:1, c0 + 1:c0 + cw + 1])
                nc.sync.dma_start(
                    out=t_ep[c0:c0 + cw, :].rearrange("t two -> () (t two)"),
                    in_=ep_g[0:1, 0:2 * cw])

            # =========== Phase B: span tiles ===========
            for grp in range(n_grp):
                giT = gitpool.tile([P, len(SK_CH), GRP * P], F32R, tag="giT")
                for ti in range(GRP):
                    t = grp * GRP + ti
                    gi = gipool.tile([P, GI], F32, tag="gi")
                    eE0 = wpool.tile([P, D_E], F32, tag="eE0")
                    eE1 = wpool.tile([P, D_E], F32, tag="eE1")
                    epr = wpool.tile([P, 2], F32, tag="epr")
                    idx_s = starts_sb[:, t:t + 1]
                    idx_e = ends_sb[:, t:t + 1]
                    wcol = widths_sb[:, t:t + 1]
                    nc.gpsimd.indirect_dma_start(
                        out=gi[:, 0:D_S], out_offset=None, in_=t_states[:],
                        in_offset=bass.IndirectOffsetOnAxis(ap=idx_s, axis=0))
                    nc.gpsimd.indirect_dma_start(
                        out=gi[:, D_S:2 * D_S], out_offset=None, in_=t_states[:],
                        in_offset=bass.IndirectOffsetOnAxis(ap=idx_e, axis=0))
                    nc.gpsimd.indirect_dma_start(
                        out=eE0[:], out_offset=None, in_=t_embeds[:],
                        in_offset=bass.IndirectOffsetOnAxis(ap=idx_s, axis=0))
                    nc.gpsimd.indirect_dma_start(
                        out=eE1[:], out_offset=None, in_=t_embeds[:],
                        in_offset=bass.IndirectOffsetOnAxis(ap=idx_e, axis=0))
                    nc.gpsimd.indirect_dma_start(
                        out=epr[:], out_offset=None, in_=t_ep[:],
                        in_offset=bass.IndirectOffsetOnAxis(ap=idx_s, axis=0))
                    # span-softmax weights
                    e1m = wpool.tile([P, 1], F32, tag="e1m")
                    ssum = wpool.tile([P, 1], F32, tag="ssum")
                    rcp = wpool.tile([P, 1], F32, tag="rcp")
                    w0 = wpool.tile([P, 1], F32, tag="w0")
                    w1 = wpool.tile([P, 1], F32, tag="w1")
                    nc.vector.tensor_tensor(out=e1m[:], in0=epr[:, 1:2],
                                            in1=wcol, op=mybir.AluOpType.mult)
                    nc.vector.tensor_tensor(out=ssum[:], in0=epr[:, 0:1],
                                            in1=e1m[:], op=mybir.AluOpType.add)
                    nc.vector.reciprocal(out=rcp[:], in_=ssum[:])
                    nc.vector.tensor_tensor(out=w0[:], in0=epr[:, 0:1],
                                            in1=rcp[:], op=mybir.AluOpType.mult)
                    nc.vector.tensor_tensor(out=w1[:], in0=e1m[:], in1=rcp[:],
                                            op=mybir.AluOpType.mult)
                    # attn embeds
                    nc.vector.tensor_scalar_mul(gi[:, 2 * D_S:2 * D_S + D_E],
                                                eE0[:], w0[:, :1])
                    nc.vector.tensor_scalar_mul(eE1[:], eE1[:], w1[:, :1])
                    nc.vector.tensor_tensor(out=gi[:, 2 * D_S:2 * D_S + D_E],
                                            in0=gi[:, 2 * D_S:2 * D_S + D_E],
                                            in1=eE1[:], op=mybir.AluOpType.add)
                    # width embedding
                    nc.vector.tensor_scalar_mul(gi[:, 2 * D_S + D_E:GI],
                                                wtd[:], wcol)
                    nc.vector.tensor_tensor(out=gi[:, 2 * D_S + D_E:GI],
                                            in0=gi[:, 2 * D_S + D_E:GI],
                                            in1=wt1[:], op=mybir.AluOpType.add)
                    # write g_i tile out
                    nc.sync.dma_start(out=t_gi[t * P:(t + 1) * P, :], in_=gi[:])
                    # transposes for the mention MLP
                    for kc, (k0, kw) in enumerate(SK_CH):
                        tp = ptp.tile([P, P], F32, space="PSUM", tag="tp")
                        nc.tensor.transpose(out=tp[:kw, :],
                                            in_=gi[:, k0:k0 + kw],
                                            identity=ident[:])
                        nc.vector.tensor_copy(
                            out=giT[0:kw, kc, ti * P:(ti + 1) * P],
                            in_=tp[0:kw, :])
                # ---- mention MLP on the 512-span group ----
                NW = GRP * P
                h1ps = [pmm.tile([P, NW], F32, space="PSUM", tag="hps0", name="hps0"),
                        pmm.tile([32, NW], F32, space="PSUM", tag="hps1", name="hps1")]
                for mc, (m0, mw) in enumerate(M_CH):
                    for kc, (k0, kw) in enumerate(SK_CH):
                        nc.tensor.matmul(out=h1ps[mc][0:mw, :],
                                         lhsT=sw1[0:kw, kc, m0:m0 + mw],
                                         rhs=giT[0:kw, kc, :],
                                         start=(kc == 0),
                                         stop=(kc == len(SK_CH) - 1))
                h1sb = hpool.tile([P, NW], F32R, tag="h1sb")
                h1sb2 = hpool.tile([32, NW], F32R, tag="h1sb2")
                nc.scalar.activation(out=h1sb[:], in_=h1ps[0][:],
                                     func=mybir.ActivationFunctionType.Relu,
                                     bias=sb1[:, 0:1])
                nc.scalar.activation(out=h1sb2[:], in_=h1ps[1][0:32, :],
                                     func=mybir.ActivationFunctionType.Relu,
                                     bias=sb1[0:32, 1:2])
                h2ps = [pmm.tile([P, NW], F32, space="PSUM", tag="h2ps0", name="h2ps0"),
                        pmm.tile([32, NW], F32, space="PSUM", tag="h2ps1", name="h2ps1")]
                for mc, (m0, mw) in enumerate(M_CH):
                    nc.tensor.matmul(out=h2ps[mc][0:mw, :],
                                     lhsT=sw2[0:P, 0, m0:m0 + mw],
                                     rhs=h1sb[:], start=True, stop=False)
                    nc.tensor.matmul(out=h2ps[mc][0:mw, :],
                                     lhsT=sw2[0:32, 1, m0:m0 + mw],
                                     rhs=h1sb2[:], start=False, stop=True)
                h2sb = hpool.tile([P, NW], F32R, tag="h2sb")
                h2sb2 = hpool.tile([32, NW], F32R, tag="h2sb2")
                nc.scalar.activation(out=h2sb[:], in_=h2ps[0][:],
                                     func=mybir.ActivationFunctionType.Relu,
                                     bias=sb2[:, 0:1])
                nc.scalar.activation(out=h2sb2[:], in_=h2ps[1][0:32, :],
                                     func=mybir.ActivationFunctionType.Relu,
                                     bias=sb2[0:32, 1:2])
                scps = pmm.tile([1, NW], F32, space="PSUM", tag="scps")
                nc.tensor.matmul(out=scps[:], lhsT=sw3[0:P, 0:1], rhs=h2sb[:],
                                 start=True, stop=False)
                nc.tensor.matmul(out=scps[:], lhsT=sw3[0:32, 1:2], rhs=h2sb2[:],
                                 start=False, stop=True)
                sc_g = wpool.tile([1, NW], F32, tag="sc_g", bufs=2)
                nc.vector.tensor_scalar_add(sc_g[:], scps[:], sb3[0:1, 0:1])
                nc.sync.dma_start(
                    out=t_sc[grp * NW:(grp + 1) * NW, :].rearrange(
                        "s o -> () (s o)"),
                    in_=sc_g[:])
    nc.compile()
    _NC_CACHE[key] = nc
    return nc


def _wrap_cols_n(arr, tiles):
    """[tiles*P] -> [P, tiles] with [p, t] = arr[t*P + p]."""
    return np.ascontiguousarray(arr.reshape(tiles, P).T)


def _wrap_cols(arr2d):
    """[S_LOC] -> [P, TILES] with [p, t] = arr[t*P + p]."""
    return _wrap_cols_n(arr2d, TILES)


def _chunk_rows(w, n_rows_pad, n_cols_pad):
    """Pad a [K, M] matrix to [n_rows_pad, n_cols_pad] and wrap rows into
    [P, n_rows_pad//P, n_cols_pad] K-chunks."""
    kp = np.zeros((n_rows_pad, n_cols_pad), np.float32)
    kp[:w.shape[0], :w.shape[1]] = w
    return np.ascontiguousarray(
        kp.reshape(n_rows_pad // P, P, n_cols_pad).transpose(1, 0, 2))


def _bias_chunks(b):
    bp = np.zeros((2 * P,), np.float32)
    bp[:b.shape[0]] = b
    return np.ascontiguousarray(bp.reshape(2, P).T)


def kernel(**inputs):
    states = np.asarray(inputs["states"], np.float32)
    embeds = np.asarray(inputs["embeds"], np.float32)
    starts = np.asarray(inputs["span_starts"], np.int64)
    widths = np.asarray(inputs["span_widths"], np.int64)
    n_t = states.shape[0]

    # token windows per core
    t_los, rngs = [], []
    for c in range(N_CORES):
        s = starts[c * S_C:(c + 1) * S_C]
        t_lo = int(s.min())
        t_hi = int(s.max()) + 1          # rows up to start+1 are touched
        t_los.append(t_lo)
        rngs.append(t_hi - t_lo + 1)
    T_LOC = -(-max(rngs) // A_GRP_T) * A_GRP_T

    nc = _build(T_LOC)

    # parameter prep (shared across cores)
    aw1w = _chunk_rows(np.asarray(inputs["aW1"], np.float32), len(AK_CH) * P, HP)
    aw2w = _chunk_rows(np.asarray(inputs["aW2"], np.float32), 2 * P, HP)
    aw3w = _chunk_rows(np.asarray(inputs["aW3"], np.float32), 2 * P, 1)[:, :, 0]
    sw1w = _chunk_rows(np.asarray(inputs["sW1"], np.float32), len(SK_CH) * P, HP)
    sw2w = _chunk_rows(np.asarray(inputs["sW2"], np.float32), 2 * P, HP)
    sw3w = _chunk_rows(np.asarray(inputs["sW3"], np.float32), 2 * P, 1)[:, :, 0]
    wtab = np.asarray(inputs["width_table"], np.float32)
    shared = {
        "aw1w": np.ascontiguousarray(aw1w),
        "aw2w": np.ascontiguousarray(aw2w),
        "aw3w": np.ascontiguousarray(aw3w),
        "sw1w": np.ascontiguousarray(sw1w),
        "sw2w": np.ascontiguousarray(sw2w),
        "sw3w": np.ascontiguousarray(sw3w),
        "ab1c": _bias_chunks(np.asarray(inputs["ab1"], np.float32)),
        "ab2c": _bias_chunks(np.asarray(inputs["ab2"], np.float32)),
        "ab3": np.asarray(inputs["ab3"], np.float32).reshape(1, 1),
        "sb1c": _bias_chunks(np.asarray(inputs["sb1"], np.float32)),
        "sb2c": _bias_chunks(np.asarray(inputs["sb2"], np.float32)),
        "sb3": np.asarray(inputs["sb3"], np.float32).reshape(1, 1),
        "wt1": np.ascontiguousarray(np.broadcast_to(wtab[1], (P, D_D))),
        "wtd": np.ascontiguousarray(
            np.broadcast_to(wtab[2] - wtab[1], (P, D_D))),
    }

    in_maps = []
    for c in range(N_CORES):
        sl = slice(c * S_C, (c + 1) * S_C)
        t_lo = t_los[c]
        hi = min(t_lo + T_LOC, n_t)
        st_loc = np.zeros((T_LOC, D_S), np.float32)
        st_loc[:hi - t_lo] = states[t_lo:hi]
        em_loc = np.zeros((T_LOC, D_E), np.float32)
        em_loc[:hi - t_lo] = embeds[t_lo:hi]
        s_loc = np.zeros((S_LOC,), np.int64)
        s_loc[:S_C] = starts[sl] - t_lo
        w_loc = np.zeros((S_LOC,), np.int64)
        w_loc[:S_C] = widths[sl]
        in_maps.append({
            "states_loc": st_loc,
            "embeds_loc": em_loc,
            "starts_w": _wrap_cols(s_loc.astype(np.int32)),
            "ends_w": _wrap_cols((s_loc + w_loc).astype(np.int32)),
            "widths_w": _wrap_cols(w_loc.astype(np.float32)),
            **shared,
        })

    res = run_bass_kernel_spmd(nc, in_maps, list(range(N_CORES)))
    LAST_RUN["nc"] = nc
    LAST_RUN["in_maps"] = in_maps

    g_i = np.concatenate(
        [res.results[c]["gi_out"][:S_C] for c in range(N_CORES)], axis=0)
    sc = np.concatenate(
        [res.results[c]["sc_out"][:S_C] for c in range(N_CORES)], axis=0)
    return g_i, sc
